# revision 5
# baseline (speedup 1.0000x reference)
import sys

for p in ("/opt/trn_rl_repo",):
    if p not in sys.path:
        sys.path.insert(0, p)

import numpy as np
import ml_dtypes

from concourse import bass, mybir, bacc, tile
from concourse.ap import AP
from concourse.bass_utils import run_bass_kernel_spmd


def _install_ntff_hook():
    try:
        from antenv import axon_hooks  # noqa: F401
        return
    except ImportError:
        pass
    import types
    try:
        import antenv
    except ImportError:
        return
    mod = types.ModuleType("antenv.axon_hooks")
    _h = {"hook": None}
    mod.set_axon_ntff_profile_hook = lambda h: _h.__setitem__("hook", h)
    mod.get_axon_ntff_profile_hook = lambda: _h["hook"]
    sys.modules["antenv.axon_hooks"] = mod
    antenv.axon_hooks = mod
    try:
        from trn_agent_boot.trn_boot import _ntff_profile_via_ctypes
        h = _ntff_profile_via_ctypes("/opt/axon/libaxon_pjrt.so")
        if h is not None:
            mod.set_axon_ntff_profile_hook(h)
    except Exception:
        pass


_install_ntff_hook()

F32 = mybir.dt.float32
F32R = mybir.dt.float32r
BF16 = mybir.dt.bfloat16
MUL = mybir.AluOpType.mult
ADD = mybir.AluOpType.add
MAX = mybir.AluOpType.max
AXX = mybir.AxisListType.X
EXP = mybir.ActivationFunctionType.Exp

B, C, H, W = 16, 256, 96, 96
CQ = 32
S = H * W          # 9216
NB = 32            # bands per direction
NCORE = 8
BPC = B // NCORE   # 2 batches per core
QKW = 72           # q(32) | k(32) | sigma(1) | pad(7)
PW = QKW + 256     # 328 proj width


def _apv(t, off, dims):
    """Custom view on a tile/tensor AP: keep partition dim, custom free dims."""
    b = t[:] if not isinstance(t, AP) else t
    part = list(b.ap[0])
    return AP(b.tensor, b.offset + off, [part] + [list(d) for d in dims])


def build_graph():
    nc = bacc.Bacc(None, target_bir_lowering=False)

    xa_e = nc.declare_dram_parameter("xa", [BPC, 2, 128, S], BF16, isOutput=False)
    wall_e = nc.declare_dram_parameter("wall", [2, 128, PW], BF16, isOutput=False)
    wqkp_e = nc.declare_dram_parameter("wqkp", [2, 128, 256], BF16, isOutput=False)
    ipat_e = nc.declare_dram_parameter("ipat", [96, 1152], F32, isOutput=False)
    pstr_e = nc.declare_dram_parameter("pstr", [96, 288], F32, isOutput=False)
    idt_e = nc.declare_dram_parameter("idt", [96, 96], F32, isOutput=False)
    gam_e = nc.declare_dram_parameter("gam", [128, 1], F32, isOutput=False)
    bvr_e = nc.declare_dram_parameter("bvr", [2, 96, 128], BF16, isOutput=False)
    out_e = nc.declare_dram_parameter("out", [BPC, 2, 128, S], BF16, isOutput=True)

    with tile.TileContext(nc) as tc:
        with (
            tc.tile_pool(name="const", bufs=1) as cp,
            tc.tile_pool(name="main", bufs=1) as mp,
            tc.tile_pool(name="work", bufs=2) as wp,
        ):
            wall_sb = []
            wqkp_sb = []
            bvr_sb = []
            for cc in range(2):
                t = cp.tile([128, PW], BF16, tag=f"wall{cc}")
                nc.sync.dma_start(t[:], wall_e[cc])
                wall_sb.append(t)
                t = cp.tile([128, 256], BF16, tag=f"wqkp{cc}")
                nc.sync.dma_start(t[:], wqkp_e[cc])
                wqkp_sb.append(t)
                t = cp.tile([96, 128], BF16, tag=f"bvr{cc}")
                nc.sync.dma_start(t[:], bvr_e[cc])
                bvr_sb.append(t)
            ipat_sb = cp.tile([96, 1152], F32, tag="ipat")
            nc.sync.dma_start(ipat_sb[:], ipat_e[:])
            pstr_sb = cp.tile([96, 288], F32, tag="pstr")
            nc.sync.dma_start(pstr_sb[:], pstr_e[:])
            idt_sb = cp.tile([96, 96], F32, tag="idt")
            nc.sync.dma_start(idt_sb[:], idt_e[:])
            gam_sb = cp.tile([128, 1], F32, tag="gam")
            nc.sync.dma_start(gam_sb[:], gam_e[:])

            for b in range(BPC):
                xa_sb = []
                for cc in range(2):
                    t = mp.tile([128, S], BF16, tag=f"xa{cc}")
                    nc.sync.dma_start(t[:], xa_e[b, cc])
                    xa_sb.append(t)

                qk_sb = mp.tile([96, 96 * QKW], BF16, tag="qk")
                v_sb = mp.tile([96, 96 * 256], BF16, tag="v")
                qkc_sb = mp.tile([96, 96 * QKW], BF16, tag="qkc")

                # ---------- projections ----------
                with tc.tile_pool(name=f"pj{b}", bufs=2, space="PSUM") as pj:
                    # H-pass: row-layout; 32 groups x 3 lines (psum slices @512)
                    for g in range(NB):
                        ps = pj.tile([96, 1536], F32, tag="pj")
                        for l3 in range(3):
                            h = 3 * g + l3
                            o = 512 * l3
                            for cc in range(2):
                                nc.tensor.matmul(
                                    _apv(ps, o, [[1, PW]]),
                                    xa_sb[cc][:, h * 96:(h + 1) * 96],
                                    wall_sb[cc][:],
                                    start=(cc == 0),
                                    stop=(cc == 1),
                                )
                        nc.vector.tensor_copy(
                            qk_sb[:, g * 3 * QKW:(g + 1) * 3 * QKW],
                            _apv(ps, 0, [[512, 3], [1, QKW]]),
                        )
                        nc.scalar.copy(
                            v_sb[:, g * 768:(g + 1) * 768],
                            _apv(ps, QKW, [[512, 3], [1, 256]]),
                        )
                    # V-pass: column-layout q|k|sigma; 16 groups x 6 lines (@256)
                    for g in range(16):
                        ps = pj.tile([96, 1536], F32, tag="pj")
                        for l6 in range(6):
                            wl = 6 * g + l6
                            o = 256 * l6
                            for cc in range(2):
                                lhs = AP(xa_sb[cc][:].tensor, xa_sb[cc][:].offset + wl,
                                         [[S, 128], [96, 96]])
                                nc.tensor.matmul(
                                    _apv(ps, o, [[1, 256]]),
                                    lhs,
                                    wqkp_sb[cc][:],
                                    start=(cc == 0),
                                    stop=(cc == 1),
                                )
                        nc.vector.tensor_copy(
                            qkc_sb[:, g * 6 * QKW:(g + 1) * 6 * QKW],
                            _apv(ps, 0, [[256, 6], [1, QKW]]),
                        )

                # ---------- scores + softmax (both directions) ----------
                def scores(src_sb, name):
                    s_raw = wp.tile([96, 288], F32, tag=f"sraw{name}")
                    for k in range(3):
                        for j in range(3):
                            prod = wp.tile([96, 1024], BF16, tag="prod")
                            nc.vector.tensor_tensor(
                                prod[:],
                                _apv(src_sb, k * QKW, [[3 * QKW, 32], [1, 32]]),
                                _apv(src_sb, j * QKW + 32, [[3 * QKW, 32], [1, 32]]),
                                MUL,
                            )
                            nc.vector.tensor_reduce(
                                s_raw[:, (3 * k + j) * 32:(3 * k + j + 1) * 32],
                                _apv(prod, 0, [[32, 32], [1, 32]]),
                                AXX, ADD,
                            )
                    # reorder to (k, n, j) + add sigma_{j-line}
                    sx = wp.tile([96, 288], F32, tag=f"sx{name}")
                    nc.vector.tensor_tensor(
                        sx[:],
                        _apv(s_raw, 0, [[96, 3], [1, 32], [32, 3]]),
                        _apv(src_sb, 64, [[0, 3], [3 * QKW, 32], [QKW, 3]]),
                        ADD,
                    )
                    m3 = wp.tile([96, 96], F32, tag=f"m3{name}")
                    nc.vector.tensor_reduce(
                        m3[:], _apv(sx, 0, [[3, 96], [1, 3]]), AXX, MAX)
                    te = wp.tile([96, 288], F32, tag=f"te{name}")
                    nc.vector.tensor_tensor(
                        _apv(te, 0, [[3, 96], [1, 3]]),
                        _apv(sx, 0, [[3, 96], [1, 3]]),
                        _apv(m3, 0, [[1, 96], [0, 3]]),
                        mybir.AluOpType.subtract,
                    )
                    nc.scalar.activation(te[:], te[:], EXP)
                    s3 = wp.tile([96, 96], F32, tag=f"s3{name}")
                    nc.vector.tensor_reduce(
                        s3[:], _apv(te, 0, [[3, 96], [1, 3]]), AXX, ADD)
                    r3 = wp.tile([96, 96], F32, tag=f"r3{name}")
                    nc.vector.reciprocal(r3[:], s3[:])
                    a_t = mp.tile([96, 288], F32, tag=f"A{name}")
                    nc.vector.tensor_tensor(
                        _apv(a_t, 0, [[3, 96], [1, 3]]),
                        _apv(te, 0, [[3, 96], [1, 3]]),
                        _apv(r3, 0, [[1, 96], [0, 3]]),
                        MUL,
                    )
                    return a_t

                a_h = scores(qk_sb, "h")   # [96 w, (k:96, n:3, j:1)]
                a_v = scores(qkc_sb, "v")  # [96 h, (k':96, m:3, j':1)]

                # B_H[n,j] = sum_k A_h  -> [96 w, (n:3, j:1)]
                b_h = mp.tile([96, 96], F32, tag="bh")
                nc.vector.tensor_reduce(
                    b_h[:], _apv(a_h, 0, [[3, 32], [1, 3], [96, 3]]), AXX, ADD)

                # A_v permute to (j', w'=3m+k') then transpose -> AvT [96 w', (j':96, h:1)]
                av_p = wp.tile([96, 288], F32, tag="avp")
                nc.vector.tensor_copy(
                    av_p[:], _apv(a_v, 0, [[1, 3], [3, 32], [96, 3]]))
                av_s = wp.tile([96, 96], F32, tag="avs")
                nc.vector.tensor_reduce(
                    av_s[:], _apv(a_v, 0, [[3, 32], [1, 3], [96, 3]]), AXX, ADD)

                avt = mp.tile([96, 288], F32, tag="avt")
                avst = mp.tile([96, 96], F32, tag="avst")
                with tc.tile_pool(name=f"tp{b}", bufs=2, space="PSUM") as tp:
                    for jp in range(3):
                        pt = tp.tile([96, 96], F32, tag="tp")
                        nc.tensor.transpose(
                            pt[:], av_p[:, jp * 96:(jp + 1) * 96], idt_sb[:])
                        nc.vector.tensor_copy(avt[:, jp * 96:(jp + 1) * 96], pt[:])
                    pt = tp.tile([96, 96], F32, tag="tp")
                    nc.tensor.transpose(pt[:], av_s[:], idt_sb[:])
                    nc.vector.tensor_copy(avst[:], pt[:])

                # ---------- AV + combine + out ----------
                with tc.tile_pool(name=f"av{b}", bufs=4, space="PSUM") as avp:
                    for n in range(NB):
                        rhs = wp.tile([96, 1152], BF16, tag="rhs")
                        # diag blocks for k=0..2 (9 blocks of 96)
                        nc.vector.tensor_tensor(
                            rhs[:, 0:864],
                            ipat_sb[:, 0:864],
                            _apv(a_h, n * 3, [[96, 3], [1, 3], [0, 96]]),
                            MUL,
                        )
                        # bias diag blocks (j=0..2)
                        nc.vector.tensor_tensor(
                            rhs[:, 864:1152],
                            ipat_sb[:, 864:1152],
                            _apv(b_h, n * 3, [[1, 3], [0, 96]]),
                            MUL,
                        )
                        # vertical attention stripes into block (k, j==k)
                        for k in range(3):
                            for jp in range(3):
                                sl = rhs[:, k * 288 + k * 96 + 0:k * 288 + k * 96 + 96] \
                                    if False else _apv(rhs, k * 288 + k * 96, [[1, 96]])
                                nc.vector.scalar_tensor_tensor(
                                    sl,
                                    pstr_sb[:, jp * 96:(jp + 1) * 96],
                                    avt[:, jp * 96 + 3 * n + k:jp * 96 + 3 * n + k + 1],
                                    sl,
                                    MUL, ADD,
                                )
                        # vertical bias stripes into bias blocks
                        for j in range(3):
                            sl = _apv(rhs, 864 + j * 96, [[1, 96]])
                            nc.vector.scalar_tensor_tensor(
                                sl,
                                ipat_sb[:, 0:96],
                                avst[:, 3 * n + j:3 * n + j + 1],
                                sl,
                                MUL, ADD,
                            )
                        for cc in range(2):
                            pso = avp.tile([128, 288], F32, tag=f"av{cc}")
                            for k in range(3):
                                nc.tensor.matmul(
                                    pso[:],
                                    _apv(v_sb, (3 * n + k) * 256 + cc * 128, [[1, 128]]),
                                    rhs[:, k * 288:(k + 1) * 288],
                                    start=(k == 0),
                                    stop=False,
                                )
                            nc.tensor.matmul(
                                pso[:], bvr_sb[cc][:], rhs[:, 864:1152],
                                start=False, stop=True,
                            )
                            nc.vector.scalar_tensor_tensor(
                                xa_sb[cc][:, n * 288:(n + 1) * 288],
                                pso[:],
                                gam_sb[:],
                                xa_sb[cc][:, n * 288:(n + 1) * 288],
                                MUL, ADD,
                            )
                            nc.sync.dma_start(
                                out_e[b, cc, :, n * 288:(n + 1) * 288],
                                xa_sb[cc][:, n * 288:(n + 1) * 288],
                            )
    nc.compile()
    return nc


def _host_prep(x, Wq, bq, Wk, bk, Wv, bv, gamma):
    x = np.ascontiguousarray(x, np.float32)
    sig_w = (bq @ Wk).astype(np.float32)          # [256]
    pad = np.zeros((7, 256), np.float32)
    wall = np.concatenate([Wq, Wk, sig_w[None], pad, Wv], 0)      # [328, 256]
    wallT = np.stack([np.ascontiguousarray(wall[:, :128].T),
                      np.ascontiguousarray(wall[:, 128:].T)])     # [2,128,328]
    wqkp = np.concatenate([Wq, Wk, sig_w[None],
                           np.zeros((191, 256), np.float32)], 0)  # [256, 256]
    wqkpT = np.stack([np.ascontiguousarray(wqkp[:, :128].T),
                      np.ascontiguousarray(wqkp[:, 128:].T)])
    ipat = np.tile(np.eye(96, dtype=np.float32), (1, 12))         # [96, 1152]
    pstr = np.zeros((96, 288), np.float32)
    for wpr in range(96):
        m = wpr // 3
        for j in range(3):
            pstr[wpr, j * 96 + 3 * m + j] = 1.0
    idt = np.eye(96, dtype=np.float32)
    gam = np.full((128, 1), float(np.asarray(gamma).reshape(-1)[0]), np.float32)
    bvr = np.stack([np.tile(bv[None, :128], (96, 1)),
                    np.tile(bv[None, 128:], (96, 1))]).astype(ml_dtypes.bfloat16)
    xr = x.reshape(B, 2, 128, S)
    in_maps = []
    for i in range(NCORE):
        in_maps.append({
            "xa": np.ascontiguousarray(xr[i * BPC:(i + 1) * BPC]).astype(ml_dtypes.bfloat16),
            "wall": wallT.astype(ml_dtypes.bfloat16), "wqkp": wqkpT.astype(ml_dtypes.bfloat16), "ipat": ipat, "pstr": pstr,
            "idt": idt, "gam": gam, "bvr": bvr,
        })
    return in_maps


_CACHE = {}


def kernel(x, Wq, bq, Wk, bk, Wv, bv, gamma, _trace=False):
    x = np.asarray(x, np.float32)
    in_maps = _host_prep(x, np.asarray(Wq, np.float32), np.asarray(bq, np.float32),
                         np.asarray(Wk, np.float32), np.asarray(bk, np.float32),
                         np.asarray(Wv, np.float32), np.asarray(bv, np.float32),
                         np.asarray(gamma, np.float32))
    if "nc" not in _CACHE:
        _CACHE["nc"] = build_graph()
    nc = _CACHE["nc"]
    res = run_bass_kernel_spmd(nc, in_maps, list(range(NCORE)), trace=_trace)
    kernel.last_result = res
    out = np.empty((B, C, H, W), np.float32)
    for i in range(NCORE):
        o = np.asarray(res.results[i]["out"], np.float32)   # [BPC, 2, 128, S]
        for b in range(BPC):
            out[i * BPC + b] = o[b].reshape(C, H, W)
    return out


if __name__ == "__main__":
    rng = np.random.default_rng(0)
    xs = {k: rng.standard_normal(s).astype(np.float32) * (0.05 if k != "x" else 1.0)
          for k, s in [("x", (B, C, H, W)), ("Wq", (CQ, C)), ("bq", (CQ,)),
                       ("Wk", (CQ, C)), ("bk", (CQ,)), ("Wv", (C, C)),
                       ("bv", (C,)), ("gamma", (1,))]}
    y = kernel(**xs)
    print("ran", y.shape)


# revision 11
# speedup vs baseline: 1.0442x; 1.0442x over previous
import sys

for p in ("/opt/trn_rl_repo",):
    if p not in sys.path:
        sys.path.insert(0, p)

import numpy as np
import ml_dtypes

from concourse import bass, mybir, bacc, tile
from concourse.ap import AP
from concourse.bass_utils import run_bass_kernel_spmd


def _install_ntff_hook():
    try:
        from antenv import axon_hooks  # noqa: F401
        return
    except ImportError:
        pass
    import types
    try:
        import antenv
    except ImportError:
        return
    mod = types.ModuleType("antenv.axon_hooks")
    _h = {"hook": None}
    mod.set_axon_ntff_profile_hook = lambda h: _h.__setitem__("hook", h)
    mod.get_axon_ntff_profile_hook = lambda: _h["hook"]
    sys.modules["antenv.axon_hooks"] = mod
    antenv.axon_hooks = mod
    try:
        from trn_agent_boot.trn_boot import _ntff_profile_via_ctypes
        h = _ntff_profile_via_ctypes("/opt/axon/libaxon_pjrt.so")
        if h is not None:
            mod.set_axon_ntff_profile_hook(h)
    except Exception:
        pass


_install_ntff_hook()

F32 = mybir.dt.float32
F32R = mybir.dt.float32r
BF16 = mybir.dt.bfloat16
MUL = mybir.AluOpType.mult
ADD = mybir.AluOpType.add
MAX = mybir.AluOpType.max
AXX = mybir.AxisListType.X
EXP = mybir.ActivationFunctionType.Exp

B, C, H, W = 16, 256, 96, 96
CQ = 32
S = H * W          # 9216
NB = 32            # bands per direction
NCORE = 8
BPC = B // NCORE   # 2 batches per core
QKW = 72           # q(32) | k(32) | sigma(1) | pad(7)
PW = QKW + 256     # 328 proj width


def _apv(t, off, dims):
    """Custom view on a tile/tensor AP: keep partition dim, custom free dims."""
    b = t[:] if not isinstance(t, AP) else t
    part = list(b.ap[0])
    return AP(b.tensor, b.offset + off, [part] + [list(d) for d in dims])


def build_graph():
    nc = bacc.Bacc(None, target_bir_lowering=False)

    xa_e = nc.declare_dram_parameter("xa", [BPC, 2, 128, S], BF16, isOutput=False)
    wall_e = nc.declare_dram_parameter("wall", [2, 128, PW], BF16, isOutput=False)
    wqkp_e = nc.declare_dram_parameter("wqkp", [2, 128, 256], BF16, isOutput=False)
    ipat_e = nc.declare_dram_parameter("ipat", [96, 1152], BF16, isOutput=False)
    pstr_e = nc.declare_dram_parameter("pstr", [96, 96], BF16, isOutput=False)
    idt_e = nc.declare_dram_parameter("idt", [96, 96], F32, isOutput=False)
    gam_e = nc.declare_dram_parameter("gam", [128, 1], F32, isOutput=False)
    bvr_e = nc.declare_dram_parameter("bvr", [2, 96, 128], BF16, isOutput=False)
    out_e = nc.declare_dram_parameter("out", [BPC, 2, 128, S], BF16, isOutput=True)

    with tile.TileContext(nc) as tc:
        with (
            tc.tile_pool(name="const", bufs=1) as cp,
            tc.tile_pool(name="main", bufs=1) as mp,
            tc.tile_pool(name="work", bufs=2) as wp,
        ):
            wall_sb = []
            wqkp_sb = []
            bvr_sb = []
            for cc in range(2):
                t = cp.tile([128, PW], BF16, tag=f"wall{cc}")
                nc.sync.dma_start(t[:], wall_e[cc])
                wall_sb.append(t)
                t = cp.tile([128, 256], BF16, tag=f"wqkp{cc}")
                nc.sync.dma_start(t[:], wqkp_e[cc])
                wqkp_sb.append(t)
                t = cp.tile([96, 128], BF16, tag=f"bvr{cc}")
                nc.sync.dma_start(t[:], bvr_e[cc])
                bvr_sb.append(t)
            ipat_sb = cp.tile([96, 1152], BF16, tag="ipat")
            nc.sync.dma_start(ipat_sb[:], ipat_e[:])
            pstr_sb = cp.tile([96, 96], BF16, tag="pstr")
            nc.sync.dma_start(pstr_sb[:], pstr_e[:])
            idt_sb = cp.tile([96, 96], F32, tag="idt")
            nc.sync.dma_start(idt_sb[:], idt_e[:])
            gam_sb = cp.tile([128, 1], F32, tag="gam")
            nc.sync.dma_start(gam_sb[:], gam_e[:])

            for b in range(BPC):
                xa_sb = []
                for cc in range(2):
                    t = mp.tile([128, S], BF16, tag=f"xa{cc}")
                    nc.sync.dma_start(t[:], xa_e[b, cc])
                    xa_sb.append(t)

                qk_sb = mp.tile([96, 96 * QKW], BF16, tag="qk")
                v_sb = mp.tile([96, 96 * 256], BF16, tag="v")
                qkc_sb = mp.tile([96, 96 * QKW], BF16, tag="qkc")

                # ---------- projections ----------
                with tc.tile_pool(name=f"pj{b}", bufs=2, space="PSUM") as pj:
                    # H-pass: row-layout; 32 groups x 3 lines (psum slices @512)
                    for g in range(NB):
                        ps = pj.tile([96, 1536], F32, tag="pj")
                        for l3 in range(3):
                            h = 3 * g + l3
                            o = 512 * l3
                            for cc in range(2):
                                nc.tensor.matmul(
                                    _apv(ps, o, [[1, PW]]),
                                    xa_sb[cc][:, h * 96:(h + 1) * 96],
                                    wall_sb[cc][:],
                                    start=(cc == 0),
                                    stop=(cc == 1),
                                )
                        nc.vector.tensor_copy(
                            qk_sb[:, g * 3 * QKW:(g + 1) * 3 * QKW],
                            _apv(ps, 0, [[512, 3], [1, QKW]]),
                        )
                        nc.scalar.copy(
                            v_sb[:, g * 768:(g + 1) * 768],
                            _apv(ps, QKW, [[512, 3], [1, 256]]),
                        )
                    # V-pass: column-layout q|k|sigma; 16 groups x 6 lines (@256)
                    for g in range(16):
                        ps = pj.tile([96, 1536], F32, tag="pj")
                        for l6 in range(6):
                            wl = 6 * g + l6
                            o = 256 * l6
                            for cc in range(2):
                                lhs = AP(xa_sb[cc][:].tensor, xa_sb[cc][:].offset + wl,
                                         [[S, 128], [96, 96]])
                                nc.tensor.matmul(
                                    _apv(ps, o, [[1, 256]]),
                                    lhs,
                                    wqkp_sb[cc][:],
                                    start=(cc == 0),
                                    stop=(cc == 1),
                                )
                        nc.scalar.copy(
                            qkc_sb[:, g * 6 * QKW:(g + 1) * 6 * QKW],
                            _apv(ps, 0, [[256, 6], [1, QKW]]),
                        )

                # ---------- scores + softmax (both directions) ----------
                def scores(src_sb, name):
                    s_raw = wp.tile([96, 288], F32, tag=f"sraw{name}")
                    for k in range(3):
                        for j in range(3):
                            prod = wp.tile([96, 1024], BF16, tag="prod")
                            nc.vector.tensor_tensor(
                                prod[:],
                                _apv(src_sb, k * QKW, [[3 * QKW, 32], [1, 32]]),
                                _apv(src_sb, j * QKW + 32, [[3 * QKW, 32], [1, 32]]),
                                MUL,
                            )
                            nc.vector.tensor_reduce(
                                s_raw[:, (3 * k + j) * 32:(3 * k + j + 1) * 32],
                                _apv(prod, 0, [[32, 32], [1, 32]]),
                                AXX, ADD,
                            )
                    # reorder to (k, n, j) + add sigma_{j-line}
                    sx = wp.tile([96, 288], F32, tag=f"sx{name}")
                    nc.gpsimd.tensor_tensor(
                        sx[:],
                        _apv(s_raw, 0, [[96, 3], [1, 32], [32, 3]]),
                        _apv(src_sb, 64, [[0, 3], [3 * QKW, 32], [QKW, 3]]),
                        ADD,
                    )
                    m3 = wp.tile([96, 96], F32, tag=f"m3{name}")
                    nc.vector.tensor_reduce(
                        m3[:], _apv(sx, 0, [[3, 96], [1, 3]]), AXX, MAX)
                    te = wp.tile([96, 288], F32, tag=f"te{name}")
                    nc.gpsimd.tensor_tensor(
                        _apv(te, 0, [[3, 96], [1, 3]]),
                        _apv(sx, 0, [[3, 96], [1, 3]]),
                        _apv(m3, 0, [[1, 96], [0, 3]]),
                        mybir.AluOpType.subtract,
                    )
                    nc.scalar.activation(te[:], te[:], EXP)
                    s3 = wp.tile([96, 96], F32, tag=f"s3{name}")
                    nc.vector.tensor_reduce(
                        s3[:], _apv(te, 0, [[3, 96], [1, 3]]), AXX, ADD)
                    r3 = wp.tile([96, 96], F32, tag=f"r3{name}")
                    nc.vector.reciprocal(r3[:], s3[:])
                    a_t = mp.tile([96, 288], BF16, tag=f"A{name}")
                    nc.gpsimd.tensor_tensor(
                        _apv(a_t, 0, [[3, 96], [1, 3]]),
                        _apv(te, 0, [[3, 96], [1, 3]]),
                        _apv(r3, 0, [[1, 96], [0, 3]]),
                        MUL,
                    )
                    return a_t

                a_h = scores(qk_sb, "h")   # [96 w, (k:96, n:3, j:1)]
                a_v = scores(qkc_sb, "v")  # [96 h, (k':96, m:3, j':1)]

                # B_H[n,j] = sum_k A_h  -> [96 w, (n:3, j:1)]
                b_h = mp.tile([96, 96], F32, tag="bh")
                nc.vector.tensor_reduce(
                    b_h[:], _apv(a_h, 0, [[3, 32], [1, 3], [96, 3]]), AXX, ADD)

                # A_v permute to (j', w'=3m+k') then transpose -> AvT [96 w', (j':96, h:1)]
                av_p = wp.tile([96, 288], F32, tag="avp")
                nc.vector.tensor_copy(
                    av_p[:], _apv(a_v, 0, [[1, 3], [3, 32], [96, 3]]))
                av_s = wp.tile([96, 96], F32, tag="avs")
                nc.vector.tensor_reduce(
                    av_s[:], _apv(a_v, 0, [[3, 32], [1, 3], [96, 3]]), AXX, ADD)

                avt = mp.tile([96, 288], BF16, tag="avt")
                avst = mp.tile([96, 96], F32, tag="avst")
                with tc.tile_pool(name=f"tp{b}", bufs=2, space="PSUM") as tp:
                    for jp in range(3):
                        pt = tp.tile([96, 96], F32, tag="tp")
                        nc.tensor.transpose(
                            pt[:], av_p[:, jp * 96:(jp + 1) * 96], idt_sb[:])
                        nc.vector.tensor_copy(avt[:, jp * 96:(jp + 1) * 96], pt[:])
                    pt = tp.tile([96, 96], F32, tag="tp")
                    nc.tensor.transpose(pt[:], av_s[:], idt_sb[:])
                    nc.vector.tensor_copy(avst[:], pt[:])

                # btot = B_H + AvsT  (same (n,j) <-> h=3n+j index layout), bf16
                btot = mp.tile([96, 96], BF16, tag="btot")
                nc.gpsimd.tensor_tensor(btot[:], b_h[:], avst[:], ADD)

                # Mv_all[w', line*96 + w] = Pfull[w', w] * AvT[w', (w%3)*96 + line]
                # one op: in0 = Pfull bcast over lines; in1 = pure AP view of AvT
                mv = mp.tile([96, 9216], BF16, tag="mv")
                nc.gpsimd.tensor_tensor(
                    _apv(mv, 0, [[96, 96], [3, 32], [1, 3]]),
                    _apv(pstr_sb, 0, [[0, 96], [3, 32], [1, 3]]),
                    _apv(avt, 0, [[1, 96], [0, 32], [96, 3]]),
                    MUL,
                )

                # ---------- AV + combine + out ----------
                with tc.tile_pool(name=f"av{b}", bufs=4, space="PSUM") as avp:
                    for n in range(NB):
                        rhs = wp.tile([96, 1152], BF16, tag="rhs")
                        beng = nc.vector if n % 2 == 0 else nc.gpsimd
                        # diag blocks for k=0..2 (9 blocks of 96)
                        beng.tensor_tensor(
                            rhs[:, 0:864],
                            ipat_sb[:, 0:864],
                            _apv(a_h, n * 3, [[96, 3], [1, 3], [0, 96]]),
                            MUL,
                        )
                        # bias diag blocks (j=0..2)
                        nc.vector.tensor_tensor(
                            rhs[:, 864:1152],
                            ipat_sb[:, 864:1152],
                            _apv(btot, n * 3, [[1, 3], [0, 96]]),
                            MUL,
                        )
                        # vertical attention: add Mv_all lines 3n+k into blocks (k, j==k)
                        nc.gpsimd.tensor_tensor(
                            _apv(rhs, 0, [[384, 3], [1, 96]]),
                            _apv(rhs, 0, [[384, 3], [1, 96]]),
                            _apv(mv, 3 * n * 96, [[96, 3], [1, 96]]),
                            ADD,
                        )
                        for cc in range(2):
                            pso = avp.tile([128, 288], F32, tag=f"av{cc}")
                            for k in range(3):
                                nc.tensor.matmul(
                                    pso[:],
                                    _apv(v_sb, (3 * n + k) * 256 + cc * 128, [[1, 128]]),
                                    rhs[:, k * 288:(k + 1) * 288],
                                    start=(k == 0),
                                    stop=False,
                                )
                            nc.tensor.matmul(
                                pso[:], bvr_sb[cc][:], rhs[:, 864:1152],
                                start=False, stop=True,
                            )
                            cmb = wp.tile([128, 288], BF16, tag=f"cmb{cc}")
                            nc.scalar.copy(cmb[:], pso[:])
                            nc.vector.scalar_tensor_tensor(
                                xa_sb[cc][:, n * 288:(n + 1) * 288],
                                cmb[:],
                                gam_sb[:],
                                xa_sb[cc][:, n * 288:(n + 1) * 288],
                                MUL, ADD,
                            )
                            nc.sync.dma_start(
                                out_e[b, cc, :, n * 288:(n + 1) * 288],
                                xa_sb[cc][:, n * 288:(n + 1) * 288],
                            )
    nc.compile()
    return nc


def _host_prep(x, Wq, bq, Wk, bk, Wv, bv, gamma):
    x = np.ascontiguousarray(x, np.float32)
    sig_w = (bq @ Wk).astype(np.float32)          # [256]
    pad = np.zeros((7, 256), np.float32)
    wall = np.concatenate([Wq, Wk, sig_w[None], pad, Wv], 0)      # [328, 256]
    wallT = np.stack([np.ascontiguousarray(wall[:, :128].T),
                      np.ascontiguousarray(wall[:, 128:].T)])     # [2,128,328]
    wqkp = np.concatenate([Wq, Wk, sig_w[None],
                           np.zeros((191, 256), np.float32)], 0)  # [256, 256]
    wqkpT = np.stack([np.ascontiguousarray(wqkp[:, :128].T),
                      np.ascontiguousarray(wqkp[:, 128:].T)])
    ipat = np.tile(np.eye(96), (1, 12)).astype(ml_dtypes.bfloat16)  # [96, 1152]
    pstr = np.kron(np.eye(32), np.ones((3, 3))).astype(ml_dtypes.bfloat16)  # [96, 96]
    idt = np.eye(96, dtype=np.float32)
    gam = np.full((128, 1), float(np.asarray(gamma).reshape(-1)[0]), np.float32)
    bvr = np.stack([np.tile(bv[None, :128], (96, 1)),
                    np.tile(bv[None, 128:], (96, 1))]).astype(ml_dtypes.bfloat16)
    xr = x.reshape(B, 2, 128, S)
    in_maps = []
    for i in range(NCORE):
        in_maps.append({
            "xa": np.ascontiguousarray(xr[i * BPC:(i + 1) * BPC]).astype(ml_dtypes.bfloat16),
            "wall": wallT.astype(ml_dtypes.bfloat16), "wqkp": wqkpT.astype(ml_dtypes.bfloat16), "ipat": ipat, "pstr": pstr,
            "idt": idt, "gam": gam, "bvr": bvr,
        })
    return in_maps


_CACHE = {}


def kernel(x, Wq, bq, Wk, bk, Wv, bv, gamma, _trace=False):
    x = np.asarray(x, np.float32)
    in_maps = _host_prep(x, np.asarray(Wq, np.float32), np.asarray(bq, np.float32),
                         np.asarray(Wk, np.float32), np.asarray(bk, np.float32),
                         np.asarray(Wv, np.float32), np.asarray(bv, np.float32),
                         np.asarray(gamma, np.float32))
    if "nc" not in _CACHE:
        _CACHE["nc"] = build_graph()
    nc = _CACHE["nc"]
    res = run_bass_kernel_spmd(nc, in_maps, list(range(NCORE)), trace=_trace)
    kernel.last_result = res
    out = np.empty((B, C, H, W), np.float32)
    for i in range(NCORE):
        o = np.asarray(res.results[i]["out"], np.float32)   # [BPC, 2, 128, S]
        for b in range(BPC):
            out[i * BPC + b] = o[b].reshape(C, H, W)
    return out


if __name__ == "__main__":
    rng = np.random.default_rng(0)
    xs = {k: rng.standard_normal(s).astype(np.float32) * (0.05 if k != "x" else 1.0)
          for k, s in [("x", (B, C, H, W)), ("Wq", (CQ, C)), ("bq", (CQ,)),
                       ("Wk", (CQ, C)), ("bk", (CQ,)), ("Wv", (C, C)),
                       ("bv", (C,)), ("gamma", (1,))]}
    y = kernel(**xs)
    print("ran", y.shape)
